# revision 12
# baseline (speedup 1.0000x reference)
"""Trainium2 Bass kernel for nn_MetaModel (moe_routing).

Math: per-ticker MLP states are linear in the M=8 mesa coefficients:
  states[t] = base + bias + meta_W @ mesa_W[:, t]
so with A[t] = [1, mesa_W[:, t]] (9 coeffs):
  w1_eff[t] = sum_m A[t,m] * W1_m,  b1_eff, w2_eff, b2_eff likewise.

Per row n (ticker t=ticker[n]), tile = 128 rows:
  ZA[n, 64(m-1)+j] = x_aug[n] @ W1aug_m[j]     m=1..8   (PE, 512 cols)
  [q | pre0]       = [A[t] | x_aug[n]] @ blockdiag(W2aug, W1aug_0)
                                                (PE, one 129-col matmul)
  pre += sum_m A[t,m] * ZA[...]                (DVE broadcast scale ->
                                                PE identity accumulate)
  h  = relu(pre)   and  qs = copy(q)           (ACT, psum -> sbuf)
  tm = h_aug * qs                              (DVE elementwise)
  out[n] = rowsum(tm)                          (ACT activation accum_out)

No indirect gathers: per-row coefficients A[t_n] are marshaled host-side
into dense tensors (AS row-major for the scale; the A rows stacked on
top of x_aug in AXT for the fused matmul).  Weight tables are host-
summed (base+bias), so there is no device phase 0.
PSUM: ZA x2 | QZ x3 = 5 banks, one tensor per bank.
Data parallel over N=32768 rows across 8 cores (4096 rows each).
"""
import sys

sys.path.insert(0, "/opt/trn_rl_repo")
import numpy as np

from concourse.bass_utils import run_bass_kernel_spmd
from concourse import bass, mybir

F32 = mybir.dt.float32
BF16 = mybir.dt.bfloat16
AF = mybir.ActivationFunctionType
ALU = mybir.AluOpType

D, H, T, M, N, S = 32, 64, 1024, 8, 32768, 2177
NCORES = 8
R = N // NCORES          # rows per core = 4096
NT = R // 128            # tiles per core = 32
KA = D + 1               # 33 (ones-augmented contraction)
KAX = KA + 9             # 42 (x_aug rows, then A rows)
ZW = 8 * H               # 512 (scaled blocks m=1..8)
W2W = H + 1              # 65
QZW = W2W + H            # 129: [q (65) | pre (64)]

# AXT chunk boundaries (in tiles): first chunk small so tile 0 starts early
CHT = [4, 13, 22, 32]

last_results = None      # test.py reads trace info from here

_cached = None


def _build_program():
    nc = bass.Bass()

    axt = nc.dram_tensor("axt", [KAX, R], BF16, kind="ExternalInput")
    wc = nc.dram_tensor("wc", [KA, ZW], BF16, kind="ExternalInput")
    wcz = nc.dram_tensor("wcz", [KAX, QZW], BF16, kind="ExternalInput")
    ass = nc.dram_tensor("ass", [128, NT * 8], BF16, kind="ExternalInput")
    ident = nc.dram_tensor("ident", [128, 128], BF16, kind="ExternalInput")
    y = nc.dram_tensor("y", [128, NT], F32, kind="ExternalOutput")

    from contextlib import ExitStack
    with ExitStack() as ctx:
        e = ctx.enter_context
        # sbuf
        AXT = e(nc.sbuf_tensor([KAX, R], BF16))
        WCs = e(nc.sbuf_tensor([KA, ZW], BF16))
        WZs = e(nc.sbuf_tensor([KAX, QZW], BF16))
        ASs = e(nc.sbuf_tensor([128, NT * 8], BF16))
        IDN = e(nc.sbuf_tensor([128, 128], BF16))
        AM = e(nc.sbuf_tensor([128, 2 * ZW], BF16))
        HB = e(nc.sbuf_tensor([128, 3 * W2W], F32))
        QS = e(nc.sbuf_tensor([128, 2 * W2W], F32))
        TMP = e(nc.sbuf_tensor([128, 2 * W2W], F32))
        DUMP = e(nc.sbuf_tensor([128, W2W], F32))
        OUT = e(nc.sbuf_tensor([128, NT], F32))
        # psum: whole banks per tensor
        ZA0 = e(nc.psum_tensor([128, ZW], F32))
        ZA1 = e(nc.psum_tensor([128, ZW], F32))
        QZ0 = e(nc.psum_tensor([128, QZW], F32))
        QZ1 = e(nc.psum_tensor([128, QZW], F32))
        QZ2 = e(nc.psum_tensor([128, QZW], F32))
        ZAP = [ZA0, ZA1]
        QZP = [QZ0, QZ1, QZ2]
        # semaphores
        s_w = e(nc.semaphore("s_w"))        # wc+wcz DMAs (sync queue)
        s_k = e(nc.semaphore("s_k"))        # ASs+IDN DMAs (gpsimd queue)
        s_x = [e(nc.semaphore(f"s_x{c}")) for c in range(4)]
        s_z = e(nc.semaphore("s_z"))        # ZA matmul done (per tile)
        s_pre = e(nc.semaphore("s_pre"))    # combine done (per tile)
        s_sc = e(nc.semaphore("s_sc"))      # scale done (per tile)
        s_h = e(nc.semaphore("s_h"))        # relu (and qcopy) done
        s_m = e(nc.semaphore("s_m"))        # w2 mult done (per tile)
        s_t = e(nc.semaphore("s_t"))        # accum reduce done (per tile)
        s_y = e(nc.semaphore("s_y"))
        block = e(nc.Block())

        @block.sync
        def _(sync):
            sync.dma_start(out=WCs[:], in_=wc[:]).then_inc(s_w, 16)
            sync.dma_start(out=WZs[:], in_=wcz[:]).then_inc(s_w, 16)
            lo = 0
            for c, hi in enumerate(CHT):
                sync.dma_start(
                    out=AXT[:, lo * 128:hi * 128], in_=axt[:, lo * 128:hi * 128]
                ).then_inc(s_x[c], 16)
                lo = hi
            sync.wait_ge(s_t, NT)
            sync.dma_start(out=y[0:43, :], in_=OUT[0:43, :]).then_inc(s_y, 16)
            sync.wait_ge(s_y, 48)

        @block.gpsimd
        def _(gp):
            gp.dma_start(out=ASs[:], in_=ass[:]).then_inc(s_k, 16)
            gp.dma_start(out=IDN[:], in_=ident[:]).then_inc(s_k, 16)
            gp.wait_ge(s_t, NT)
            gp.dma_start(out=y[43:86, :], in_=OUT[43:86, :]).then_inc(s_y, 16)

        @block.tensor
        def _(te):
            for i in range(NT + 1):
                if i < NT:
                    if i == 0:
                        te.wait_ge(s_w, 32)
                        te.wait_ge(s_x[0], 16)
                    elif i == CHT[0]:
                        te.wait_ge(s_x[1], 16)
                    elif i == CHT[1]:
                        te.wait_ge(s_x[2], 16)
                    elif i == CHT[2]:
                        te.wait_ge(s_x[3], 16)
                    # ZA[i%2] free is implied: previous iteration's combine
                    # waited s_sc >= i-1 (scale(i-2) done) already.
                    nc.tensor.matmul(ZAP[i % 2][:],
                                     lhsT=AXT[0:KA, i * 128:(i + 1) * 128],
                                     rhs=WCs[:],
                                     start=True, stop=True).then_inc(s_z, 1)
                    if i >= 3:
                        te.wait_ge(s_h, i - 2)    # QZ[i%3] free (relu(i-3))
                    nc.tensor.matmul(QZP[i % 3][:],
                                     lhsT=AXT[:, i * 128:(i + 1) * 128],
                                     rhs=WZs[:], start=True, stop=False,
                                     skip_group_check=True)
                if i >= 1:
                    if i == 1:
                        te.wait_ge(s_k, 32)       # IDN loaded
                    pq = (i - 1) % 3
                    te.wait_ge(s_sc, i)           # scale(i-1) done
                    for m in range(8):
                        op = nc.tensor.matmul(
                            QZP[pq][:, W2W:QZW], lhsT=IDN[:],
                            rhs=AM[:, ((i - 1) % 2) * ZW + m * H:
                                   ((i - 1) % 2) * ZW + (m + 1) * H],
                            start=False, stop=(m == 7),
                            skip_group_check=True)
                    op.then_inc(s_pre, 1)

        @block.vector
        def _(ve):
            nc.vector.memset(HB[:, H:H + 1], 1.0)
            nc.vector.memset(HB[:, W2W + H:W2W + H + 1], 1.0)
            nc.vector.memset(HB[:, 2 * W2W + H:2 * W2W + H + 1], 1.0)
            ve.wait_ge(s_k, 16)  # ASs loaded
            for i in range(NT + 2):
                j = i - 2
                if j >= 0:
                    ve.wait_ge(s_h, j + 1)        # relu(j)+qcopy(j) done
                    if j >= 2:
                        ve.wait_ge(s_t, j - 1)    # TMP[j%2] free (accred(j-2))
                    nc.vector.tensor_tensor(
                        out=TMP[:, (j % 2) * W2W:(j % 2 + 1) * W2W],
                        in0=HB[:, (j % 3) * W2W:(j % 3 + 1) * W2W],
                        in1=QS[:, (j % 2) * W2W:(j % 2 + 1) * W2W],
                        op=ALU.mult,
                    ).then_inc(s_m, 1)
                if i < NT:
                    ve.wait_ge(s_z, i + 1)        # ZA(i) done
                    if i >= 2:
                        ve.wait_ge(s_pre, i - 1)  # AM[i%2] free (combine(i-2))
                    nc.vector.tensor_tensor(
                        out=AM[:, (i % 2) * ZW:(i % 2 + 1) * ZW].rearrange(
                            "p (m j) -> p m j", j=H),
                        in0=ZAP[i % 2][:].rearrange("p (m j) -> p m j", j=H),
                        in1=ASs[:, i * 8:(i + 1) * 8].unsqueeze(2)
                        .broadcast_to((128, 8, H)),
                        op=ALU.mult,
                    ).then_inc(s_sc, 1)

        @block.scalar
        def _(act):
            for i in range(NT + 2):
                j = i - 2
                if j >= 0:
                    act.wait_ge(s_m, j + 1)       # mult(j) done
                    nc.scalar.activation(
                        out=DUMP[:],
                        in_=TMP[:, (j % 2) * W2W:(j % 2 + 1) * W2W],
                        func=AF.Copy,
                        accum_out=OUT[:, j:j + 1],
                    ).then_inc(s_t, 1)
                if i < NT:
                    act.wait_ge(s_pre, i + 1)     # combine(i) done (same bank)
                    if i >= 2:
                        act.wait_ge(s_m, i - 1)   # QS[i%2] free (mult(i-2))
                    nc.scalar.activation(
                        out=QS[:, (i % 2) * W2W:(i % 2 + 1) * W2W],
                        in_=QZP[i % 3][:, 0:W2W], func=AF.Copy,
                    )
                    nc.scalar.activation(
                        out=HB[:, (i % 3) * W2W: (i % 3) * W2W + H],
                        in_=QZP[i % 3][:, W2W:QZW],
                        func=AF.Relu,
                    ).then_inc(s_h, 1)
                if i == NT + 1:
                    act.wait_ge(s_t, NT)
                    act.dma_start(out=y[86:128, :],
                                  in_=OUT[86:128, :]).then_inc(s_y, 16)

    return nc


def _host_prep(x, ticker, mesa_w, meta_w, meta_b, base):
    f32 = np.float32
    import ml_dtypes
    bf16 = ml_dtypes.bfloat16

    # 9-basis state stack: m=0 -> base + bias, m=1..8 -> meta_W columns
    st = np.zeros((9, S), f32)
    st[0] = base + meta_b
    st[1:] = meta_w.T

    # wc: [33, 512] — blocks m=1..8 of [w1_m.T ; b1_m]
    wcf = np.zeros((KA, ZW), f32)
    for m in range(1, 9):
        c0 = (m - 1) * H
        blk = st[m, :H * D].reshape(H, D)
        wcf[0:D, c0:c0 + H] = blk.T
        wcf[D, c0:c0 + H] = st[m, H * D:H * D + H]
    wc = wcf.astype(bf16)

    # wcz: [42, 129] block-diag: rows 0..32 x cols 65..128 = [w1_0.T ; b1_0],
    # rows 33..41 x cols 0..64 = W2aug basis
    wzf = np.zeros((KAX, QZW), f32)
    blk0 = st[0, :H * D].reshape(H, D)
    wzf[0:D, W2W:W2W + H] = blk0.T
    wzf[D, W2W:W2W + H] = st[0, H * D:H * D + H]
    wzf[KA:KA + 9, 0:H] = st[:, H * D + H:H * D + H + H]   # w2 basis
    wzf[KA:KA + 9, H] = st[:, S - 1]                       # b2 basis
    wcz = wzf.astype(bf16)

    ident = np.eye(128, dtype=bf16)

    # per-row mesa coefficients, [8, N] f32
    Arows = mesa_w[:, ticker]                     # [8, N]

    shared = dict(wc=wc, wcz=wcz, ident=ident)
    in_maps = []
    for c in range(NCORES):
        rows = slice(c * R, (c + 1) * R)
        axtc = np.empty((KAX, R), bf16)
        axtc[0:D] = x[rows].T
        axtc[D] = 1.0
        axtc[KA] = 1.0
        axtc[KA + 1:KAX] = Arows[:, rows]
        assc = np.ascontiguousarray(
            Arows[:, rows].reshape(8, NT, 128).transpose(2, 1, 0)
            .reshape(128, NT * 8)).astype(bf16)
        in_maps.append(dict(axt=np.ascontiguousarray(axtc),
                            ass=assc, **shared))
    return in_maps


def kernel(x, ticker, mesa_layer_weight, meta_layer_weight, meta_layer_bias,
           base_state):
    global _cached, last_results
    if _cached is None:
        _cached = _build_program()
    nc = _cached
    in_maps = _host_prep(
        np.asarray(x, np.float32), np.asarray(ticker),
        np.asarray(mesa_layer_weight, np.float32),
        np.asarray(meta_layer_weight, np.float32),
        np.asarray(meta_layer_bias, np.float32),
        np.asarray(base_state, np.float32))
    res = run_bass_kernel_spmd(nc, in_maps, core_ids=list(range(NCORES)))
    last_results = res
    out = np.empty((N, 1), np.float32)
    for c in range(NCORES):
        yc = res.results[c]["y"]              # [128, NT]
        out[c * R:(c + 1) * R, 0] = yc.T.reshape(R)
    return out


# revision 14
# speedup vs baseline: 1.0699x; 1.0699x over previous
"""Trainium2 Bass kernel for nn_MetaModel (moe_routing).

Math: per-ticker MLP states are linear in the M=8 mesa coefficients:
  states[t] = base + bias + meta_W @ mesa_W[:, t]
so with A[t] = [1, mesa_W[:, t]] (9 coeffs):
  w1_eff[t] = sum_m A[t,m] * W1_m,  b1_eff, w2_eff, b2_eff likewise.

Per row n (ticker t=ticker[n]), tile = 128 rows:
  ZA[n, 64(m-1)+j] = x_aug[n] @ W1aug_m[j]     m=1..8   (PE, 512 cols)
  [q | pre0]       = [A[t] | x_aug[n]] @ blockdiag(W2aug, W1aug_0)
                                                (PE, one 129-col matmul)
  pre += sum_m A[t,m] * ZA[...]                (DVE broadcast scale ->
                                                PE identity accumulate)
  h  = relu(pre)   and  qs = copy(q)           (ACT, psum -> sbuf)
  tm = h_aug * qs                              (DVE elementwise)
  out[n] = rowsum(tm)                          (ACT activation accum_out)

No indirect gathers: per-row coefficients A[t_n] are marshaled host-side
into dense tensors (AS row-major for the scale; the A rows stacked on
top of x_aug in AXT for the fused matmul).  Weight tables are host-
summed (base+bias), so there is no device phase 0.
PSUM: ZA x2 | QZ x3 = 5 banks, one tensor per bank.
Data parallel over N=32768 rows across 8 cores (4096 rows each).
"""
import sys

sys.path.insert(0, "/opt/trn_rl_repo")
import numpy as np

from concourse.bass_utils import run_bass_kernel_spmd
from concourse import bass, mybir

F32 = mybir.dt.float32
BF16 = mybir.dt.bfloat16
AF = mybir.ActivationFunctionType
ALU = mybir.AluOpType

D, H, T, M, N, S = 32, 64, 1024, 8, 32768, 2177
NCORES = 8
R = N // NCORES          # rows per core = 4096
NT = R // 128            # tiles per core = 32
KA = D + 1               # 33 (ones-augmented contraction)
KAX = KA + 9             # 42 (x_aug rows, then A rows)
ZW = 8 * H               # 512 (scaled blocks m=1..8)
W2W = H + 1              # 65
QZW = W2W + H            # 129: [q (65) | pre (64)]

# AXT chunk boundaries (in tiles): first chunk small so tile 0 starts early
CHT = [4, 13, 22, 32]

last_results = None      # test.py reads trace info from here

_cached = None


def _build_program():
    nc = bass.Bass()

    axt = nc.dram_tensor("axt", [KAX, R], BF16, kind="ExternalInput")
    wc = nc.dram_tensor("wc", [KA, ZW], BF16, kind="ExternalInput")
    wcz = nc.dram_tensor("wcz", [KAX, QZW], BF16, kind="ExternalInput")
    ass = nc.dram_tensor("ass", [128, NT * 8], BF16, kind="ExternalInput")
    ident = nc.dram_tensor("ident", [128, 128], BF16, kind="ExternalInput")
    y = nc.dram_tensor("y", [128, NT], F32, kind="ExternalOutput")

    from contextlib import ExitStack
    with ExitStack() as ctx:
        e = ctx.enter_context
        # sbuf
        AXT = e(nc.sbuf_tensor([KAX, R], BF16))
        WCs = e(nc.sbuf_tensor([KA, ZW], BF16))
        WZs = e(nc.sbuf_tensor([KAX, QZW], BF16))
        ASs = e(nc.sbuf_tensor([128, NT * 8], BF16))
        IDN = e(nc.sbuf_tensor([128, 128], BF16))
        AM = e(nc.sbuf_tensor([128, 2 * ZW], BF16))
        HB = e(nc.sbuf_tensor([128, 3 * W2W], F32))
        TMP = e(nc.sbuf_tensor([128, 2 * W2W], F32))
        DUMP = e(nc.sbuf_tensor([128, W2W], F32))
        OUT = e(nc.sbuf_tensor([128, NT], F32))
        # psum: whole banks per tensor
        ZA0 = e(nc.psum_tensor([128, ZW], F32))
        ZA1 = e(nc.psum_tensor([128, ZW], F32))
        QZ0 = e(nc.psum_tensor([128, QZW], F32))
        QZ1 = e(nc.psum_tensor([128, QZW], F32))
        QZ2 = e(nc.psum_tensor([128, QZW], F32))
        ZAP = [ZA0, ZA1]
        QZP = [QZ0, QZ1, QZ2]
        # semaphores
        s_w = e(nc.semaphore("s_w"))        # wc+wcz DMAs (sync queue)
        s_k = e(nc.semaphore("s_k"))        # ASs+IDN DMAs (gpsimd queue)
        s_x = [e(nc.semaphore(f"s_x{c}")) for c in range(4)]
        s_z = e(nc.semaphore("s_z"))        # ZA matmul done (per tile)
        s_pre = e(nc.semaphore("s_pre"))    # combine done (per tile)
        s_sc = e(nc.semaphore("s_sc"))      # scale done (per tile)
        s_h = e(nc.semaphore("s_h"))        # relu (and qcopy) done
        s_m = e(nc.semaphore("s_m"))        # w2 mult done (per tile)
        s_t = e(nc.semaphore("s_t"))        # accum reduce done (per tile)
        s_y = e(nc.semaphore("s_y"))
        block = e(nc.Block())

        @block.sync
        def _(sync):
            lo = 0
            for c, hi in enumerate(CHT):
                sync.dma_start(
                    out=AXT[:, lo * 128:hi * 128], in_=axt[:, lo * 128:hi * 128]
                ).then_inc(s_x[c], 16)
                lo = hi
            sync.wait_ge(s_t, 28)
            sync.dma_start(out=y[:, 0:24], in_=OUT[:, 0:24]).then_inc(s_y, 16)
            sync.wait_ge(s_t, NT)
            sync.dma_start(out=y[:, 24:NT], in_=OUT[:, 24:NT]).then_inc(s_y, 16)
            sync.wait_ge(s_y, 32)

        @block.gpsimd
        def _(gp):
            gp.dma_start(out=WCs[:], in_=wc[:]).then_inc(s_w, 16)
            gp.dma_start(out=WZs[:], in_=wcz[:]).then_inc(s_w, 16)
            gp.dma_start(out=ASs[:], in_=ass[:]).then_inc(s_k, 16)
            gp.dma_start(out=IDN[:], in_=ident[:]).then_inc(s_k, 16)

        @block.tensor
        def _(te):
            for i in range(NT + 1):
                if i < NT:
                    if i == 0:
                        te.wait_ge(s_w, 32)
                        te.wait_ge(s_x[0], 16)
                    elif i == CHT[0]:
                        te.wait_ge(s_x[1], 16)
                    elif i == CHT[1]:
                        te.wait_ge(s_x[2], 16)
                    elif i == CHT[2]:
                        te.wait_ge(s_x[3], 16)
                    # ZA[i%2] free is implied: previous iteration's combine
                    # waited s_sc >= i-1 (scale(i-2) done) already.
                    nc.tensor.matmul(ZAP[i % 2][:],
                                     lhsT=AXT[0:KA, i * 128:(i + 1) * 128],
                                     rhs=WCs[:],
                                     start=True, stop=True).then_inc(s_z, 1)
                    if i >= 3:
                        te.wait_ge(s_h, i - 2)    # QZ[i%3] free (relu(i-3))
                        te.wait_ge(s_m, i - 2)    # ... and mult(i-3)
                    nc.tensor.matmul(QZP[i % 3][:],
                                     lhsT=AXT[:, i * 128:(i + 1) * 128],
                                     rhs=WZs[:], start=True, stop=False,
                                     skip_group_check=True)
                if i >= 1:
                    if i == 1:
                        te.wait_ge(s_k, 32)       # IDN loaded
                    pq = (i - 1) % 3
                    te.wait_ge(s_sc, i)           # scale(i-1) done
                    for m in range(8):
                        op = nc.tensor.matmul(
                            QZP[pq][:, W2W:QZW], lhsT=IDN[:],
                            rhs=AM[:, ((i - 1) % 2) * ZW + m * H:
                                   ((i - 1) % 2) * ZW + (m + 1) * H],
                            start=False, stop=(m == 7),
                            skip_group_check=True)
                    op.then_inc(s_pre, 1)

        @block.vector
        def _(ve):
            nc.vector.memset(HB[:, H:H + 1], 1.0)
            nc.vector.memset(HB[:, W2W + H:W2W + H + 1], 1.0)
            nc.vector.memset(HB[:, 2 * W2W + H:2 * W2W + H + 1], 1.0)
            ve.wait_ge(s_k, 16)  # ASs loaded
            for i in range(NT + 2):
                j = i - 2
                if i < NT:
                    ve.wait_ge(s_z, i + 1)        # ZA(i) done
                    if i >= 2:
                        ve.wait_ge(s_pre, i - 1)  # AM[i%2] free (combine(i-2))
                    nc.vector.tensor_tensor(
                        out=AM[:, (i % 2) * ZW:(i % 2 + 1) * ZW].rearrange(
                            "p (m j) -> p m j", j=H),
                        in0=ZAP[i % 2][:].rearrange("p (m j) -> p m j", j=H),
                        in1=ASs[:, i * 8:(i + 1) * 8].unsqueeze(2)
                        .broadcast_to((128, 8, H)),
                        op=ALU.mult,
                    ).then_inc(s_sc, 1)
                if j >= 0:
                    ve.wait_ge(s_h, j + 1)        # relu(j) done
                    if j >= 2:
                        ve.wait_ge(s_t, j - 1)    # TMP[j%2] free (accred(j-2))
                    nc.vector.tensor_tensor(
                        out=TMP[:, (j % 2) * W2W:(j % 2 + 1) * W2W],
                        in0=HB[:, (j % 3) * W2W:(j % 3 + 1) * W2W],
                        in1=QZP[j % 3][:, 0:W2W],
                        op=ALU.mult,
                    ).then_inc(s_m, 1)

        @block.scalar
        def _(act):
            for i in range(NT + 2):
                j = i - 2
                if j >= 0:
                    act.wait_ge(s_m, j + 1)       # mult(j) done
                    nc.scalar.activation(
                        out=DUMP[:],
                        in_=TMP[:, (j % 2) * W2W:(j % 2 + 1) * W2W],
                        func=AF.Copy,
                        accum_out=OUT[:, j:j + 1],
                    ).then_inc(s_t, 1)
                if i < NT:
                    act.wait_ge(s_pre, i + 1)     # combine(i) done (same bank)
                    nc.scalar.activation(
                        out=HB[:, (i % 3) * W2W: (i % 3) * W2W + H],
                        in_=QZP[i % 3][:, W2W:QZW],
                        func=AF.Relu,
                    ).then_inc(s_h, 1)

    return nc


def _host_prep(x, ticker, mesa_w, meta_w, meta_b, base):
    f32 = np.float32
    import ml_dtypes
    bf16 = ml_dtypes.bfloat16

    # 9-basis state stack: m=0 -> base + bias, m=1..8 -> meta_W columns
    st = np.zeros((9, S), f32)
    st[0] = base + meta_b
    st[1:] = meta_w.T

    # wc: [33, 512] — blocks m=1..8 of [w1_m.T ; b1_m]
    wcf = np.zeros((KA, ZW), f32)
    for m in range(1, 9):
        c0 = (m - 1) * H
        blk = st[m, :H * D].reshape(H, D)
        wcf[0:D, c0:c0 + H] = blk.T
        wcf[D, c0:c0 + H] = st[m, H * D:H * D + H]
    wc = wcf.astype(bf16)

    # wcz: [42, 129] block-diag: rows 0..32 x cols 65..128 = [w1_0.T ; b1_0],
    # rows 33..41 x cols 0..64 = W2aug basis
    wzf = np.zeros((KAX, QZW), f32)
    blk0 = st[0, :H * D].reshape(H, D)
    wzf[0:D, W2W:W2W + H] = blk0.T
    wzf[D, W2W:W2W + H] = st[0, H * D:H * D + H]
    wzf[KA:KA + 9, 0:H] = st[:, H * D + H:H * D + H + H]   # w2 basis
    wzf[KA:KA + 9, H] = st[:, S - 1]                       # b2 basis
    wcz = wzf.astype(bf16)

    ident = np.eye(128, dtype=bf16)

    # per-row mesa coefficients, [8, N] f32
    Arows = mesa_w[:, ticker]                     # [8, N]

    shared = dict(wc=wc, wcz=wcz, ident=ident)
    in_maps = []
    for c in range(NCORES):
        rows = slice(c * R, (c + 1) * R)
        axtc = np.empty((KAX, R), bf16)
        axtc[0:D] = x[rows].T
        axtc[D] = 1.0
        axtc[KA] = 1.0
        axtc[KA + 1:KAX] = Arows[:, rows]
        assc = np.ascontiguousarray(
            Arows[:, rows].reshape(8, NT, 128).transpose(2, 1, 0)
            .reshape(128, NT * 8)).astype(bf16)
        in_maps.append(dict(axt=np.ascontiguousarray(axtc),
                            ass=assc, **shared))
    return in_maps


def kernel(x, ticker, mesa_layer_weight, meta_layer_weight, meta_layer_bias,
           base_state):
    global _cached, last_results
    if _cached is None:
        _cached = _build_program()
    nc = _cached
    in_maps = _host_prep(
        np.asarray(x, np.float32), np.asarray(ticker),
        np.asarray(mesa_layer_weight, np.float32),
        np.asarray(meta_layer_weight, np.float32),
        np.asarray(meta_layer_bias, np.float32),
        np.asarray(base_state, np.float32))
    res = run_bass_kernel_spmd(nc, in_maps, core_ids=list(range(NCORES)))
    last_results = res
    out = np.empty((N, 1), np.float32)
    for c in range(NCORES):
        yc = res.results[c]["y"]              # [128, NT]
        out[c * R:(c + 1) * R, 0] = yc.T.reshape(R)
    return out


# revision 15
# speedup vs baseline: 1.0710x; 1.0011x over previous
"""Trainium2 Bass kernel for nn_MetaModel (moe_routing).

Math: per-ticker MLP states are linear in the M=8 mesa coefficients:
  states[t] = base + bias + meta_W @ mesa_W[:, t]
so with A[t] = [1, mesa_W[:, t]] (9 coeffs):
  w1_eff[t] = sum_m A[t,m] * W1_m,  b1_eff, w2_eff, b2_eff likewise.

Per row n (ticker t=ticker[n]), tile = 128 rows:
  ZA[n, 64(m-1)+j] = x_aug[n] @ W1aug_m[j]     m=1..8   (PE, 512 cols)
  [q | pre0]       = [A[t] | x_aug[n]] @ blockdiag(W2aug, W1aug_0)
                                                (PE, one 129-col matmul)
  pre += sum_m A[t,m] * ZA[...]                (DVE broadcast scale ->
                                                PE identity accumulate)
  h  = relu(pre)   and  qs = copy(q)           (ACT, psum -> sbuf)
  tm = h_aug * qs                              (DVE elementwise)
  out[n] = rowsum(tm)                          (ACT activation accum_out)

No indirect gathers: per-row coefficients A[t_n] are marshaled host-side
into dense tensors (AS row-major for the scale; the A rows stacked on
top of x_aug in AXT for the fused matmul).  Weight tables are host-
summed (base+bias), so there is no device phase 0.
PSUM: ZA x2 | QZ x3 = 5 banks, one tensor per bank.
Data parallel over N=32768 rows across 8 cores (4096 rows each).
"""
import sys

sys.path.insert(0, "/opt/trn_rl_repo")
import numpy as np

from concourse.bass_utils import run_bass_kernel_spmd
from concourse import bass, mybir

F32 = mybir.dt.float32
BF16 = mybir.dt.bfloat16
AF = mybir.ActivationFunctionType
ALU = mybir.AluOpType

D, H, T, M, N, S = 32, 64, 1024, 8, 32768, 2177
NCORES = 8
R = N // NCORES          # rows per core = 4096
NT = R // 128            # tiles per core = 32
KA = D + 1               # 33 (ones-augmented contraction)
KAX = KA + 9             # 42 (x_aug rows, then A rows)
ZW = 8 * H               # 512 (scaled blocks m=1..8)
W2W = H + 1              # 65
QZW = W2W + H            # 129: [q (65) | pre (64)]

# AXT chunk boundaries (in tiles): first chunk small so tile 0 starts early
CHT = [2, 12, 22, 32]

last_results = None      # test.py reads trace info from here

_cached = None


def _build_program():
    nc = bass.Bass()

    axt = nc.dram_tensor("axt", [KAX, R], BF16, kind="ExternalInput")
    wz = nc.dram_tensor("wz", [KAX, ZW + QZW], BF16, kind="ExternalInput")
    ass = nc.dram_tensor("ass", [128, NT * 8], BF16, kind="ExternalInput")
    ident = nc.dram_tensor("ident", [128, 128], BF16, kind="ExternalInput")
    y = nc.dram_tensor("y", [128, NT], F32, kind="ExternalOutput")

    from contextlib import ExitStack
    with ExitStack() as ctx:
        e = ctx.enter_context
        # sbuf
        AXT = e(nc.sbuf_tensor([KAX, R], BF16))
        WZ = e(nc.sbuf_tensor([KAX, ZW + QZW], BF16))
        ASs = e(nc.sbuf_tensor([128, NT * 8], BF16))
        IDN = e(nc.sbuf_tensor([128, 128], BF16))
        AM = e(nc.sbuf_tensor([128, 2 * ZW], BF16))
        HB = e(nc.sbuf_tensor([128, 3 * W2W], F32))
        TMP = e(nc.sbuf_tensor([128, 2 * W2W], F32))
        DUMP = e(nc.sbuf_tensor([128, W2W], F32))
        OUT = e(nc.sbuf_tensor([128, NT], F32))
        # psum: whole banks per tensor
        ZA0 = e(nc.psum_tensor([128, ZW], F32))
        ZA1 = e(nc.psum_tensor([128, ZW], F32))
        QZ0 = e(nc.psum_tensor([128, QZW], F32))
        QZ1 = e(nc.psum_tensor([128, QZW], F32))
        QZ2 = e(nc.psum_tensor([128, QZW], F32))
        ZAP = [ZA0, ZA1]
        QZP = [QZ0, QZ1, QZ2]
        # semaphores
        s_w = e(nc.semaphore("s_w"))        # wc+wcz DMAs (sync queue)
        s_k = e(nc.semaphore("s_k"))        # ASs+IDN DMAs (gpsimd queue)
        s_x = [e(nc.semaphore(f"s_x{c}")) for c in range(4)]
        s_z = e(nc.semaphore("s_z"))        # ZA matmul done (per tile)
        s_pre = e(nc.semaphore("s_pre"))    # combine done (per tile)
        s_sc = e(nc.semaphore("s_sc"))      # scale done (per tile)
        s_h = e(nc.semaphore("s_h"))        # relu (and qcopy) done
        s_m = e(nc.semaphore("s_m"))        # w2 mult done (per tile)
        s_t = e(nc.semaphore("s_t"))        # accum reduce done (per tile)
        s_y = e(nc.semaphore("s_y"))
        block = e(nc.Block())

        @block.sync
        def _(sync):
            lo = 0
            for c, hi in enumerate(CHT):
                sync.dma_start(
                    out=AXT[:, lo * 128:hi * 128], in_=axt[:, lo * 128:hi * 128]
                ).then_inc(s_x[c], 16)
                lo = hi
            sync.wait_ge(s_t, 28)
            sync.dma_start(out=y[:, 0:24], in_=OUT[:, 0:24]).then_inc(s_y, 16)
            sync.wait_ge(s_t, NT)
            sync.dma_start(out=y[:, 24:NT], in_=OUT[:, 24:NT]).then_inc(s_y, 16)
            sync.wait_ge(s_y, 32)

        @block.gpsimd
        def _(gp):
            gp.dma_start(out=WZ[:], in_=wz[:]).then_inc(s_w, 16)
            gp.dma_start(out=ASs[:], in_=ass[:]).then_inc(s_k, 16)
            gp.dma_start(out=IDN[:], in_=ident[:]).then_inc(s_k, 16)

        @block.tensor
        def _(te):
            for i in range(NT + 1):
                if i < NT:
                    if i == 0:
                        te.wait_ge(s_w, 16)
                        te.wait_ge(s_x[0], 16)
                    elif i == CHT[0]:
                        te.wait_ge(s_x[1], 16)
                    elif i == CHT[1]:
                        te.wait_ge(s_x[2], 16)
                    elif i == CHT[2]:
                        te.wait_ge(s_x[3], 16)
                    # ZA[i%2] free is implied: previous iteration's combine
                    # waited s_sc >= i-1 (scale(i-2) done) already.
                    nc.tensor.matmul(ZAP[i % 2][:],
                                     lhsT=AXT[0:KA, i * 128:(i + 1) * 128],
                                     rhs=WZ[0:KA, 0:ZW],
                                     start=True, stop=True).then_inc(s_z, 1)
                    if i >= 3:
                        te.wait_ge(s_h, i - 2)    # QZ[i%3] free (relu(i-3))
                        te.wait_ge(s_m, i - 2)    # ... and mult(i-3)
                    nc.tensor.matmul(QZP[i % 3][:],
                                     lhsT=AXT[:, i * 128:(i + 1) * 128],
                                     rhs=WZ[:, ZW:ZW + QZW], start=True, stop=False,
                                     skip_group_check=True)
                if i >= 1:
                    if i == 1:
                        te.wait_ge(s_k, 32)       # IDN loaded
                    pq = (i - 1) % 3
                    te.wait_ge(s_sc, i)           # scale(i-1) done
                    for m in range(8):
                        op = nc.tensor.matmul(
                            QZP[pq][:, W2W:QZW], lhsT=IDN[:],
                            rhs=AM[:, ((i - 1) % 2) * ZW + m * H:
                                   ((i - 1) % 2) * ZW + (m + 1) * H],
                            start=False, stop=(m == 7),
                            skip_group_check=True)
                    op.then_inc(s_pre, 1)

        @block.vector
        def _(ve):
            nc.vector.memset(HB[:, H:H + 1], 1.0)
            nc.vector.memset(HB[:, W2W + H:W2W + H + 1], 1.0)
            nc.vector.memset(HB[:, 2 * W2W + H:2 * W2W + H + 1], 1.0)
            ve.wait_ge(s_k, 16)  # ASs loaded
            for i in range(NT + 2):
                j = i - 2
                if i < NT:
                    ve.wait_ge(s_z, i + 1)        # ZA(i) done
                    if i >= 2:
                        ve.wait_ge(s_pre, i - 1)  # AM[i%2] free (combine(i-2))
                    nc.vector.tensor_tensor(
                        out=AM[:, (i % 2) * ZW:(i % 2 + 1) * ZW].rearrange(
                            "p (m j) -> p m j", j=H),
                        in0=ZAP[i % 2][:].rearrange("p (m j) -> p m j", j=H),
                        in1=ASs[:, i * 8:(i + 1) * 8].unsqueeze(2)
                        .broadcast_to((128, 8, H)),
                        op=ALU.mult,
                    ).then_inc(s_sc, 1)
                if j >= 0:
                    ve.wait_ge(s_h, j + 1)        # relu(j) done
                    if j >= 2:
                        ve.wait_ge(s_t, j - 1)    # TMP[j%2] free (accred(j-2))
                    nc.vector.tensor_tensor(
                        out=TMP[:, (j % 2) * W2W:(j % 2 + 1) * W2W],
                        in0=HB[:, (j % 3) * W2W:(j % 3 + 1) * W2W],
                        in1=QZP[j % 3][:, 0:W2W],
                        op=ALU.mult,
                    ).then_inc(s_m, 1)

        @block.scalar
        def _(act):
            for i in range(NT + 2):
                j = i - 2
                if j >= 0:
                    act.wait_ge(s_m, j + 1)       # mult(j) done
                    nc.scalar.activation(
                        out=DUMP[:],
                        in_=TMP[:, (j % 2) * W2W:(j % 2 + 1) * W2W],
                        func=AF.Copy,
                        accum_out=OUT[:, j:j + 1],
                    ).then_inc(s_t, 1)
                if i < NT:
                    act.wait_ge(s_pre, i + 1)     # combine(i) done (same bank)
                    nc.scalar.activation(
                        out=HB[:, (i % 3) * W2W: (i % 3) * W2W + H],
                        in_=QZP[i % 3][:, W2W:QZW],
                        func=AF.Relu,
                    ).then_inc(s_h, 1)

    return nc


def _host_prep(x, ticker, mesa_w, meta_w, meta_b, base):
    f32 = np.float32
    import ml_dtypes
    bf16 = ml_dtypes.bfloat16

    # 9-basis state stack: m=0 -> base + bias, m=1..8 -> meta_W columns
    st = np.zeros((9, S), f32)
    st[0] = base + meta_b
    st[1:] = meta_w.T

    # wz: [42, 641] — cols 0..511: blocks m=1..8 of [w1_m.T ; b1_m] (rows 0..32)
    #                cols 512..640: blockdiag(W2aug basis, W1aug_0)
    wzall = np.zeros((KAX, ZW + QZW), f32)
    wcf = wzall[:KA, :ZW]
    for m in range(1, 9):
        c0 = (m - 1) * H
        blk = st[m, :H * D].reshape(H, D)
        wcf[0:D, c0:c0 + H] = blk.T
        wcf[D, c0:c0 + H] = st[m, H * D:H * D + H]

    wzf = wzall[:, ZW:]
    blk0 = st[0, :H * D].reshape(H, D)
    wzf[0:D, W2W:W2W + H] = blk0.T
    wzf[D, W2W:W2W + H] = st[0, H * D:H * D + H]
    wzf[KA:KA + 9, 0:H] = st[:, H * D + H:H * D + H + H]   # w2 basis
    wzf[KA:KA + 9, H] = st[:, S - 1]                       # b2 basis
    wz = wzall.astype(bf16)

    ident = np.eye(128, dtype=bf16)

    # per-row mesa coefficients, [8, N] f32
    Arows = mesa_w[:, ticker]                     # [8, N]

    shared = dict(wz=wz, ident=ident)
    in_maps = []
    for c in range(NCORES):
        rows = slice(c * R, (c + 1) * R)
        axtc = np.empty((KAX, R), bf16)
        axtc[0:D] = x[rows].T
        axtc[D] = 1.0
        axtc[KA] = 1.0
        axtc[KA + 1:KAX] = Arows[:, rows]
        assc = np.ascontiguousarray(
            Arows[:, rows].reshape(8, NT, 128).transpose(2, 1, 0)
            .reshape(128, NT * 8)).astype(bf16)
        in_maps.append(dict(axt=np.ascontiguousarray(axtc),
                            ass=assc, **shared))
    return in_maps


def kernel(x, ticker, mesa_layer_weight, meta_layer_weight, meta_layer_bias,
           base_state):
    global _cached, last_results
    if _cached is None:
        _cached = _build_program()
    nc = _cached
    in_maps = _host_prep(
        np.asarray(x, np.float32), np.asarray(ticker),
        np.asarray(mesa_layer_weight, np.float32),
        np.asarray(meta_layer_weight, np.float32),
        np.asarray(meta_layer_bias, np.float32),
        np.asarray(base_state, np.float32))
    res = run_bass_kernel_spmd(nc, in_maps, core_ids=list(range(NCORES)))
    last_results = res
    out = np.empty((N, 1), np.float32)
    for c in range(NCORES):
        yc = res.results[c]["y"]              # [128, NT]
        out[c * R:(c + 1) * R, 0] = yc.T.reshape(R)
    return out


# revision 16
# speedup vs baseline: 1.1216x; 1.0472x over previous
"""Trainium2 Bass kernel for nn_MetaModel (moe_routing).

Math: per-ticker MLP states are linear in the M=8 mesa coefficients:
  states[t] = base + bias + meta_W @ mesa_W[:, t]
so with A[t] = [1, mesa_W[:, t]] (9 coeffs):
  w1_eff[t] = sum_m A[t,m] * W1_m,  b1_eff, w2_eff, b2_eff likewise.

Per row n (ticker t=ticker[n]), tile = 128 rows:
  ZA[n, 64(m-1)+j] = x_aug[n] @ W1aug_m[j]     m=1..8   (PE, 512 cols)
  [q | pre0]       = [A[t] | x_aug[n]] @ blockdiag(W2aug, W1aug_0)
                                                (PE, one 129-col matmul)
  pre += sum_m A[t,m] * ZA[...]                (DVE broadcast scale ->
                                                PE identity accumulate)
  h  = relu(pre)   and  qs = copy(q)           (ACT, psum -> sbuf)
  tm = h_aug * qs                              (DVE elementwise)
  out[n] = rowsum(tm)                          (ACT activation accum_out)

No indirect gathers: per-row coefficients A[t_n] are marshaled host-side
into dense tensors (AS row-major for the scale; the A rows stacked on
top of x_aug in AXT for the fused matmul).  Weight tables are host-
summed (base+bias), so there is no device phase 0.
PSUM: ZA x2 | QZ x3 = 5 banks, one tensor per bank.
Data parallel over N=32768 rows across 8 cores (4096 rows each).
"""
import sys

sys.path.insert(0, "/opt/trn_rl_repo")
import numpy as np

from concourse.bass_utils import run_bass_kernel_spmd
from concourse import bass, mybir

F32 = mybir.dt.float32
BF16 = mybir.dt.bfloat16
AF = mybir.ActivationFunctionType
ALU = mybir.AluOpType

D, H, T, M, N, S = 32, 64, 1024, 8, 32768, 2177
NCORES = 8
R = N // NCORES          # rows per core = 4096
NT = R // 128            # tiles per core = 32
KA = D + 1               # 33 (ones-augmented contraction)
KAX = KA + 9             # 42 (x_aug rows, then A rows)
ZW = 8 * H               # 512 (scaled blocks m=1..8)
W2W = H + 1              # 65
QZW = W2W + H            # 129: [q (65) | pre (64)]

# AXT chunk boundaries (in tiles): first chunk small so tile 0 starts early
CHT = [2, 12, 22, 32]

last_results = None      # test.py reads trace info from here

_cached = None


def _build_program():
    nc = bass.Bass()

    axt = nc.dram_tensor("axt", [KAX, R], BF16, kind="ExternalInput")
    wz = nc.dram_tensor("wz", [KAX, ZW + QZW], BF16, kind="ExternalInput")
    ass = nc.dram_tensor("ass", [128, NT * 8], BF16, kind="ExternalInput")
    ident = nc.dram_tensor("ident", [128, 128], BF16, kind="ExternalInput")
    y = nc.dram_tensor("y", [128, NT], F32, kind="ExternalOutput")

    from contextlib import ExitStack
    with ExitStack() as ctx:
        e = ctx.enter_context
        # sbuf
        AXT = e(nc.sbuf_tensor([KAX, R], BF16))
        WZ = e(nc.sbuf_tensor([KAX, ZW + QZW], BF16))
        ASs = e(nc.sbuf_tensor([128, NT * 8], BF16))
        IDN = e(nc.sbuf_tensor([128, 128], BF16))
        AM = e(nc.sbuf_tensor([128, 3 * ZW], BF16))
        HB = e(nc.sbuf_tensor([128, 4 * W2W], F32))
        TMP = e(nc.sbuf_tensor([128, 2 * W2W], F32))
        DUMP = e(nc.sbuf_tensor([128, W2W], F32))
        OUT = e(nc.sbuf_tensor([128, NT], F32))
        # psum: whole banks per tensor
        ZA0 = e(nc.psum_tensor([128, ZW], F32))
        ZA1 = e(nc.psum_tensor([128, ZW], F32))
        QZ0 = e(nc.psum_tensor([128, QZW], F32))
        QZ1 = e(nc.psum_tensor([128, QZW], F32))
        QZ2 = e(nc.psum_tensor([128, QZW], F32))
        QZ3 = e(nc.psum_tensor([128, QZW], F32))
        ZAP = [ZA0, ZA1]
        QZP = [QZ0, QZ1, QZ2, QZ3]
        # semaphores
        s_w = e(nc.semaphore("s_w"))        # wc+wcz DMAs (sync queue)
        s_k = e(nc.semaphore("s_k"))        # ASs+IDN DMAs (gpsimd queue)
        s_x = [e(nc.semaphore(f"s_x{c}")) for c in range(4)]
        s_z = e(nc.semaphore("s_z"))        # ZA matmul done (per tile)
        s_pre = e(nc.semaphore("s_pre"))    # combine done (per tile)
        s_sc = e(nc.semaphore("s_sc"))      # scale done (per tile)
        s_h = e(nc.semaphore("s_h"))        # relu (and qcopy) done
        s_m = e(nc.semaphore("s_m"))        # w2 mult done (per tile)
        s_t = e(nc.semaphore("s_t"))        # accum reduce done (per tile)
        s_y = e(nc.semaphore("s_y"))
        block = e(nc.Block())

        @block.sync
        def _(sync):
            lo = 0
            for c, hi in enumerate(CHT):
                sync.dma_start(
                    out=AXT[:, lo * 128:hi * 128], in_=axt[:, lo * 128:hi * 128]
                ).then_inc(s_x[c], 16)
                lo = hi
            sync.wait_ge(s_t, 28)
            sync.dma_start(out=y[:, 0:24], in_=OUT[:, 0:24]).then_inc(s_y, 16)
            sync.wait_ge(s_t, NT)
            sync.dma_start(out=y[:, 24:NT], in_=OUT[:, 24:NT]).then_inc(s_y, 16)
            sync.wait_ge(s_y, 32)

        @block.gpsimd
        def _(gp):
            gp.dma_start(out=WZ[:], in_=wz[:]).then_inc(s_w, 16)
            gp.dma_start(out=ASs[:], in_=ass[:]).then_inc(s_k, 16)
            gp.dma_start(out=IDN[:], in_=ident[:]).then_inc(s_k, 16)

        @block.tensor
        def _(te):
            for i in range(NT + 1):
                if i < NT:
                    if i == 0:
                        te.wait_ge(s_w, 16)
                        te.wait_ge(s_x[0], 16)
                    elif i == CHT[0]:
                        te.wait_ge(s_x[1], 16)
                    elif i == CHT[1]:
                        te.wait_ge(s_x[2], 16)
                    elif i == CHT[2]:
                        te.wait_ge(s_x[3], 16)
                    # ZA[i%2] free is implied: previous iteration's combine
                    # waited s_sc >= i-1 (scale(i-2) done) already.
                    nc.tensor.matmul(ZAP[i % 2][:],
                                     lhsT=AXT[0:KA, i * 128:(i + 1) * 128],
                                     rhs=WZ[0:KA, 0:ZW],
                                     start=True, stop=True).then_inc(s_z, 1)
                    if i >= 4:
                        te.wait_ge(s_h, i - 3)    # QZ[i%4] free (relu(i-4))
                        te.wait_ge(s_m, i - 3)    # ... and mult(i-4)
                    nc.tensor.matmul(QZP[i % 4][:],
                                     lhsT=AXT[:, i * 128:(i + 1) * 128],
                                     rhs=WZ[:, ZW:ZW + QZW], start=True, stop=False,
                                     skip_group_check=True)
                if i >= 1:
                    if i == 1:
                        te.wait_ge(s_k, 32)       # IDN loaded
                    pq = (i - 1) % 4
                    te.wait_ge(s_sc, i)           # scale(i-1) done
                    for m in range(8):
                        op = nc.tensor.matmul(
                            QZP[pq][:, W2W:QZW], lhsT=IDN[:],
                            rhs=AM[:, ((i - 1) % 3) * ZW + m * H:
                                   ((i - 1) % 3) * ZW + (m + 1) * H],
                            start=False, stop=(m == 7),
                            skip_group_check=True)
                    op.then_inc(s_pre, 1)

        @block.vector
        def _(ve):
            nc.vector.memset(HB[:, H:H + 1], 1.0)
            nc.vector.memset(HB[:, W2W + H:W2W + H + 1], 1.0)
            nc.vector.memset(HB[:, 2 * W2W + H:2 * W2W + H + 1], 1.0)
            nc.vector.memset(HB[:, 3 * W2W + H:3 * W2W + H + 1], 1.0)
            ve.wait_ge(s_k, 16)  # ASs loaded
            for i in range(NT + 2):
                j = i - 2
                if i < NT:
                    ve.wait_ge(s_z, i + 1)        # ZA(i) done
                    if i >= 3:
                        ve.wait_ge(s_pre, i - 2)  # AM[i%3] free (combine(i-3))
                    nc.vector.tensor_tensor(
                        out=AM[:, (i % 3) * ZW:(i % 3 + 1) * ZW].rearrange(
                            "p (m j) -> p m j", j=H),
                        in0=ZAP[i % 2][:].rearrange("p (m j) -> p m j", j=H),
                        in1=ASs[:, i * 8:(i + 1) * 8].unsqueeze(2)
                        .broadcast_to((128, 8, H)),
                        op=ALU.mult,
                    ).then_inc(s_sc, 1)
                if j >= 0:
                    ve.wait_ge(s_h, j + 1)        # relu(j) done
                    if j >= 2:
                        ve.wait_ge(s_t, j - 1)    # TMP[j%2] free (accred(j-2))
                    nc.vector.tensor_tensor(
                        out=TMP[:, (j % 2) * W2W:(j % 2 + 1) * W2W],
                        in0=HB[:, (j % 4) * W2W:(j % 4 + 1) * W2W],
                        in1=QZP[j % 4][:, 0:W2W],
                        op=ALU.mult,
                    ).then_inc(s_m, 1)

        @block.scalar
        def _(act):
            for i in range(NT + 2):
                j = i - 2
                if j >= 0:
                    act.wait_ge(s_m, j + 1)       # mult(j) done
                    nc.scalar.activation(
                        out=DUMP[:],
                        in_=TMP[:, (j % 2) * W2W:(j % 2 + 1) * W2W],
                        func=AF.Copy,
                        accum_out=OUT[:, j:j + 1],
                    ).then_inc(s_t, 1)
                if i < NT:
                    act.wait_ge(s_pre, i + 1)     # combine(i) done (same bank)
                    nc.scalar.activation(
                        out=HB[:, (i % 4) * W2W: (i % 4) * W2W + H],
                        in_=QZP[i % 4][:, W2W:QZW],
                        func=AF.Relu,
                    ).then_inc(s_h, 1)

    return nc


def _host_prep(x, ticker, mesa_w, meta_w, meta_b, base):
    f32 = np.float32
    import ml_dtypes
    bf16 = ml_dtypes.bfloat16

    # 9-basis state stack: m=0 -> base + bias, m=1..8 -> meta_W columns
    st = np.zeros((9, S), f32)
    st[0] = base + meta_b
    st[1:] = meta_w.T

    # wz: [42, 641] — cols 0..511: blocks m=1..8 of [w1_m.T ; b1_m] (rows 0..32)
    #                cols 512..640: blockdiag(W2aug basis, W1aug_0)
    wzall = np.zeros((KAX, ZW + QZW), f32)
    wcf = wzall[:KA, :ZW]
    for m in range(1, 9):
        c0 = (m - 1) * H
        blk = st[m, :H * D].reshape(H, D)
        wcf[0:D, c0:c0 + H] = blk.T
        wcf[D, c0:c0 + H] = st[m, H * D:H * D + H]

    wzf = wzall[:, ZW:]
    blk0 = st[0, :H * D].reshape(H, D)
    wzf[0:D, W2W:W2W + H] = blk0.T
    wzf[D, W2W:W2W + H] = st[0, H * D:H * D + H]
    wzf[KA:KA + 9, 0:H] = st[:, H * D + H:H * D + H + H]   # w2 basis
    wzf[KA:KA + 9, H] = st[:, S - 1]                       # b2 basis
    wz = wzall.astype(bf16)

    ident = np.eye(128, dtype=bf16)

    # per-row mesa coefficients, [8, N] f32
    Arows = mesa_w[:, ticker]                     # [8, N]

    shared = dict(wz=wz, ident=ident)
    in_maps = []
    for c in range(NCORES):
        rows = slice(c * R, (c + 1) * R)
        axtc = np.empty((KAX, R), bf16)
        axtc[0:D] = x[rows].T
        axtc[D] = 1.0
        axtc[KA] = 1.0
        axtc[KA + 1:KAX] = Arows[:, rows]
        assc = np.ascontiguousarray(
            Arows[:, rows].reshape(8, NT, 128).transpose(2, 1, 0)
            .reshape(128, NT * 8)).astype(bf16)
        in_maps.append(dict(axt=np.ascontiguousarray(axtc),
                            ass=assc, **shared))
    return in_maps


def kernel(x, ticker, mesa_layer_weight, meta_layer_weight, meta_layer_bias,
           base_state):
    global _cached, last_results
    if _cached is None:
        _cached = _build_program()
    nc = _cached
    in_maps = _host_prep(
        np.asarray(x, np.float32), np.asarray(ticker),
        np.asarray(mesa_layer_weight, np.float32),
        np.asarray(meta_layer_weight, np.float32),
        np.asarray(meta_layer_bias, np.float32),
        np.asarray(base_state, np.float32))
    res = run_bass_kernel_spmd(nc, in_maps, core_ids=list(range(NCORES)))
    last_results = res
    out = np.empty((N, 1), np.float32)
    for c in range(NCORES):
        yc = res.results[c]["y"]              # [128, NT]
        out[c * R:(c + 1) * R, 0] = yc.T.reshape(R)
    return out
